# revision 17
# baseline (speedup 1.0000x reference)
"""Fused multi-head attention on 8 TRN2 NeuronCores — v3.

Problem: x[2,2048,1024] -> q,k,v = x@W.T+b (16 heads x 64), softmax(q k^T/8) v,
then out @ Wp.T + bp.

Sharding: data-parallel over batch (2) x tensor-parallel over heads (4 ranks x
4 heads = 256 dims, Megatron-style).  Core c handles batch c//4, head-rank c%4.
The proj partial sums are reduced on the host (numpy); the v-bias and proj-bias
fold into one host-side vector bp_eff = bv @ Wp.T + bp.

v3 structure (trace-driven):
  - exp on ACT is the steady-state pace (~1.2us per [128,1024] block); all
    projection work runs as deadline-scheduled fillers under it.
  - PV trails exp by a lag schedule (16 early -> 4 late).  A long early lag
    postpones the v-projection + PV deadline pressure out of the congested
    stream head; the ramp-down keeps the tail short.  po/pd PSUM slots
    support any constant-or-decreasing lag (closeout of stream S is always
    emitted one position before stream S+1's first PV).
  - Denominators: exp tiles pair-summed, pairs quad-summed (blocks 0-11) on
    DVE/gpsimd, then ones-matmul pairs into pd (col-tiled concurrent).
  - x is fetched in kt-interleaved 512-token chunks (one descriptor, 8KB
    contiguous per partition); dummy matmuls warm the PE HAM clock during
    the DMA wait.
  - Output staged in bf16, one batched DMA per out-proj step; tail uses the
    scalar HWDGE queue alongside sync.
"""

import numpy as np

DIM = 1024
N_TOK = 2048
N_HEADS_LOC = 4       # heads per core
D_LOC = 256           # local q/k/v dims per core
SCALE = 64 ** -0.5
P = 128
CH = 512              # token chunk (moving free dim)
NCH = N_TOK // CH     # 4
KT = DIM // P         # 8 contraction tiles
MB = N_TOK // P       # 16 key blocks
N_CORES = 8

SEQ = [(0, 0), (1, 0), (2, 0), (0, 1), (1, 1), (2, 1), (3, 0), (3, 1)]

_NC_CACHE = {}


def build_nc(dt_mm_name="bfloat16"):
    import concourse.mybir as mybir
    import concourse.tile as tile
    from concourse import bacc
    from concourse.bass import ts

    f32 = mybir.dt.float32
    dt_mm = getattr(mybir.dt, dt_mm_name)
    Exp = mybir.ActivationFunctionType.Exp

    nc = bacc.Bacc("TRN2", target_bir_lowering=False, debug=False,
                   num_devices=N_CORES)
    xTc = nc.dram_tensor("xTc", [NCH, P, KT * CH], dt_mm,
                         kind="ExternalInput").ap()
    wqT = nc.dram_tensor("wqT", [P, KT * D_LOC], dt_mm, kind="ExternalInput").ap()
    wkT = nc.dram_tensor("wkT", [P, KT * D_LOC], dt_mm, kind="ExternalInput").ap()
    wvT = nc.dram_tensor("wvT", [P, KT * D_LOC], dt_mm, kind="ExternalInput").ap()
    wpT = nc.dram_tensor("wpT", [D_LOC, DIM], dt_mm, kind="ExternalInput").ap()
    bqk = nc.dram_tensor("bqk", [P, 4], f32, kind="ExternalInput").ap()
    outT = nc.dram_tensor("outT", [DIM, N_TOK], dt_mm, kind="ExternalOutput").ap()

    with tile.TileContext(nc) as tc:
        with (
            tc.tile_pool(name="const", bufs=1) as const,
            tc.tile_pool(name="work", bufs=2) as work,
            tc.tile_pool(name="psum", bufs=3, space="PSUM") as psum,
            tc.tile_pool(name="psum_o", bufs=2, space="PSUM") as psum_o,
        ):
            # ---- persistent SBUF state ----
            w_tiles = {name: const.tile([P, KT, D_LOC], dt_mm, tag=f"w{name}",
                                        name=f"w{name}")
                       for name in ("k", "q", "v")}
            x_all = const.tile([P, KT, N_TOK], dt_mm, tag="xall", name="xall")
            bqk_sb = const.tile([P, 4], f32, tag="bqk", name="bqk")
            wp_sb = [const.tile([P, DIM], dt_mm, tag=f"wp{i}", name=f"wp{i}")
                     for i in range(D_LOC // P)]

            def x_chunk_dma(c):
                nc.sync.dma_start(
                    out=x_all[:, :, ts(c, CH)],
                    in_=xTc[c, :, :].rearrange("p (k t) -> p k t", k=KT))

            # chunk 0 in two parallel descriptors (kt 0-3 / 4-7) to shorten
            # the prelude critical path
            for hk in range(2):
                nc.sync.dma_start(
                    out=x_all[:, 4 * hk:4 * hk + 4, ts(0, CH)],
                    in_=xTc[0, :, 4 * hk * CH:(4 * hk + 4) * CH].rearrange(
                        "p (k t) -> p k t", k=4))
            nc.sync.dma_start(out=w_tiles["k"][:],
                              in_=wkT.rearrange("p (k n) -> p k n", k=KT))
            nc.sync.dma_start(out=bqk_sb[:], in_=bqk)
            nc.sync.dma_start(out=w_tiles["q"][:],
                              in_=wqT.rearrange("p (k n) -> p k n", k=KT))
            nc.sync.dma_start(out=w_tiles["v"][:],
                              in_=wvT.rearrange("p (k n) -> p k n", k=KT))
            x_chunk_dma(1)
            x_chunk_dma(2)
            x_chunk_dma(3)
            for i in range(D_LOC // P):
                nc.sync.dma_start(out=wp_sb[i][:], in_=wpT[ts(i, P), :])

            w_sb = {name: [w_tiles[name][:, i, :] for i in range(KT)]
                    for name in ("k", "q", "v")}
            bias_sb = {"q": [bqk_sb[:, 0:1], bqk_sb[:, 1:2]],
                       "k": [bqk_sb[:, 2:3], bqk_sb[:, 3:4]]}

            ones_sb = const.tile([P, 64], dt_mm, tag="ones")
            nc.vector.memset(ones_sb[:], 1.0)

            # HAM warmup: keep the PE busy ~4us during the x DMA wait so the
            # projection matmuls run at 2.4GHz.  No data deps beyond ones_sb.
            wps = psum.tile([P, 1024], f32, tag="ps", name="warm")
            for _ in range(48):
                nc.tensor.matmul(wps[0:64, 0:64], lhsT=ones_sb[:, 0:64],
                                 rhs=ones_sb[:, 0:64])

            qk_sb = {name: [const.tile([P, N_TOK], dt_mm, tag=f"{name}T{mt}",
                                       name=f"{name}T{mt}")
                            for mt in range(D_LOC // P)]
                     for name in ("q", "k")}
            vpk_sb = [const.tile([P, N_HEADS_LOC, 64], dt_mm, tag=f"vp{nt}",
                                 name=f"vp{nt}")
                      for nt in range(MB)]
            at_sb = {}

            # ---- filler step generators (one PE matmul per yield) ----
            def kq_step(name, mt, c):
                ps = psum.tile([P, 1024], f32, tag="ps",
                               name=f"ps_{name}{mt}{c}")
                for kt in range(KT):
                    nc.tensor.matmul(
                        ps[:, 0:CH],
                        lhsT=w_sb[name][kt][:, ts(mt, P)],
                        rhs=x_all[:, kt, ts(c, CH)],
                        start=(kt == 0), stop=(kt == KT - 1),
                    )
                    yield
                nc.vector.tensor_scalar_add(
                    qk_sb[name][mt][:, ts(c, CH)], ps[:, 0:CH],
                    bias_sb[name][mt])

            def v_step(nt):
                ps = psum.tile([P, 1024], f32, tag="ps", name=f"ps_v{nt}")
                for kt in range(KT):
                    nc.tensor.matmul(
                        ps[:, 0:D_LOC],
                        lhsT=x_all[:, kt, ts(nt, P)],
                        rhs=w_sb["v"][kt][:],
                        start=(kt == 0), stop=(kt == KT - 1),
                    )
                    yield
                nc.vector.tensor_copy(vpk_sb[nt][:], ps[:, 0:D_LOC])

            def out_step(c, mp, tail=False):
                pp = psum.tile([P, 1024], f32, tag="ps", name=f"pp{c}{mp}")
                at_tiles = at_sb[c]
                for dt_i in range(2):
                    for half in range(2):
                        mo = 2 * mp + half
                        nc.tensor.matmul(
                            pp[:, ts(half, CH)],
                            lhsT=wp_sb[dt_i][:, ts(mo, P)],
                            rhs=at_tiles[dt_i][:],
                            start=(dt_i == 0), stop=(dt_i == 1),
                        )
                        yield
                os_sb = work.tile([P, 1024], dt_mm, tag="os", bufs=4,
                                  name=f"os{c}{mp}")
                if tail:
                    # split the copy across DVE and ACT (both idle) and DMA
                    # each half on its own HWDGE queue
                    nc.vector.tensor_copy(os_sb[:, 0:CH], pp[:, 0:CH])
                    nc.scalar.copy(os_sb[:, CH:1024], pp[:, CH:1024])
                    for half in range(2):
                        mo = 2 * mp + half
                        q = nc.sync if half == 0 else nc.scalar
                        q.dma_start(out=outT[ts(mo, P), ts(c, CH)],
                                    in_=os_sb[:, ts(half, CH)])
                else:
                    nc.vector.tensor_copy(os_sb[:], pp[:])
                    nc.sync.dma_start(
                        out=outT[2 * mp * P:(2 * mp + 2) * P,
                                 ts(c, CH)].rearrange("(m p) t -> p m t", p=P),
                        in_=os_sb[:].rearrange("p (m t) -> p m t", m=2))

            # ---- deadline-scheduled filler queue ----
            fillers = []   # [deadline, earliest, generator]

            def add_filler(deadline, earliest, gen):
                fillers.append([deadline, earliest, gen])
                fillers.sort(key=lambda f: f[0])

            for j in (1, 2, 3):                      # k mt0 chunks
                add_filler(4 * j - 2, j - 1, kq_step("k", 0, j))
            add_filler(14, 0, kq_step("q", 0, 1))
            for nt in range(MB):                     # v blocks
                add_filler(nt + 12, max(0, (nt // 4) * 2), v_step(nt))
            add_filler(30, 10, kq_step("q", 0, 2))
            for j in range(NCH):                     # k mt1 chunks
                add_filler(45 + 4 * j, 30 + j, kq_step("k", 1, j))
            add_filler(44, 30, kq_step("q", 1, 0))
            add_filler(60, 40, kq_step("q", 1, 1))
            add_filler(76, 50, kq_step("q", 1, 2))
            add_filler(92, 40, kq_step("q", 0, 3))
            add_filler(108, 60, kq_step("q", 1, 3))

            def pump(gen, n=None):
                if n is None:
                    for _ in gen:
                        pass
                    return False
                for _ in range(n):
                    if next(gen, "END") == "END":
                        return False
                return True

            def emit_fillers(g):
                budget = 5 if g < 32 else (3 if g < 48 else 2)
                while fillers:
                    dl, ea, gen = fillers[0]
                    if dl <= g + 1:
                        pump(gen)
                        fillers.pop(0)
                        continue
                    if ea > g or dl > g + 16 or budget <= 0:
                        break
                    if not pump(gen, budget):
                        fillers.pop(0)
                    budget = 0

            # ---- prelude: minimal warmup for the exp stream ----
            pump(kq_step("k", 0, 0))
            pump(kq_step("q", 0, 0))

            # ---- the stream ----
            blocks = [(i, c, h, mb) for i, (c, h) in enumerate(SEQ)
                      for mb in range(MB)]
            NB = len(blocks)
            pts = {}          # position -> exp tile
            pairs = {}        # (c,h) -> list of [pos, n_blocks, tile]
            pend = {}         # (c,h) -> list of [pos, tile] ready for pd
            po_pd = {}
            den_started = {}
            add_eng = [0]

            def lag_target(g):
                return 16 if g < 88 else max(4, 16 - (g - 88) // 2)

            def process_pv(gp, g):
                i2, c2, h2, mb2 = blocks[gp]
                key = (c2, h2)
                if mb2 == 0:
                    po_pd[key] = (
                        psum_o.tile([P, CH], f32, tag="po", name=f"po{c2}{h2}"),
                        psum_o.tile([P, CH], f32, tag="po", name=f"pd{c2}{h2}"),
                    )
                    pairs[key] = []
                    pend[key] = []
                    den_started[key] = False
                po, pd = po_pd[key]
                pt = pts[gp]
                st = (mb2 == 0)
                sp = (mb2 == MB - 1)
                nc.tensor.matmul(
                    po[0:64, :], lhsT=vpk_sb[mb2][:, 2 * h2, :],
                    rhs=pt[:, 0:CH], start=st, stop=sp,
                )
                nc.tensor.matmul(
                    po[64:P, :], lhsT=vpk_sb[mb2][:, 2 * h2 + 1, :],
                    rhs=pt[:, CH:1024], start=st, stop=sp,
                )
                # level-1 pair sums (blocks 14,15 stay raw for the drain)
                if mb2 % 2 == 1 and mb2 <= MB - 3:
                    pt0 = pts.pop(gp - 1)
                    ps2 = work.tile([P, 1024], dt_mm, tag="pts2", bufs=8,
                                    name=f"pts2_{c2}{h2}{mb2}")
                    if mb2 == MB - 3:
                        eng = nc.vector
                    else:
                        eng = nc.vector if add_eng[0] % 2 == 0 else nc.gpsimd
                        add_eng[0] += 1
                    eng.tensor_add(ps2[:], pt0[:], pt[:])
                    pts.pop(gp)
                    if mb2 <= 11:
                        pairs[key].append([g, 2, ps2])
                    else:
                        pend[key].append([g, ps2])
                # level-2 quad sums on DVE (inputs may be gpsimd-made; wait
                # 2 positions so their 2.5us latency never stalls DVE)
                if len(pairs[key]) >= 2 and pairs[key][1][0] <= g - 2:
                    g0, n0, t0 = pairs[key].pop(0)
                    g1, n1, t1 = pairs[key].pop(0)
                    qd = work.tile([P, 1024], dt_mm, tag="quad", bufs=4,
                                   name=f"qd{c2}{h2}{mb2}")
                    nc.vector.tensor_add(qd[:], t0[:], t1[:])
                    pend[key].append([g, qd])
                # denominator matmul pair, lagged behind its sum
                if pend[key] and pend[key][0][0] <= g - 2 and not sp:
                    _, s2 = pend[key].pop(0)
                    nc.tensor.matmul(
                        pd[0:64, :], lhsT=ones_sb[:], rhs=s2[:, 0:CH],
                        start=not den_started[key], stop=False,
                    )
                    nc.tensor.matmul(
                        pd[64:P, :], lhsT=ones_sb[:], rhs=s2[:, CH:1024],
                        start=not den_started[key], stop=False,
                    )
                    den_started[key] = True
                if sp:
                    for g0, n0, t0 in pairs.pop(key):
                        pend[key].append([g0, t0])
                    for _, s2 in pend.pop(key):
                        nc.tensor.matmul(
                            pd[0:64, :], lhsT=ones_sb[:], rhs=s2[:, 0:CH],
                            start=not den_started[key], stop=False,
                        )
                        nc.tensor.matmul(
                            pd[64:P, :], lhsT=ones_sb[:], rhs=s2[:, CH:1024],
                            start=not den_started[key], stop=False,
                        )
                        den_started[key] = True
                    for gx in (gp - 1, gp):
                        ptx = pts.pop(gx)
                        nc.tensor.matmul(
                            pd[0:64, :], lhsT=ones_sb[:], rhs=ptx[:, 0:CH],
                            start=False, stop=(gx == gp),
                        )
                        nc.tensor.matmul(
                            pd[64:P, :], lhsT=ones_sb[:], rhs=ptx[:, CH:1024],
                            start=False, stop=(gx == gp),
                        )
                    del po_pd[key]
                    rec = work.tile([P, CH], f32, tag="rec", bufs=4,
                                    name=f"rec{c2}{h2}")
                    nc.vector.reciprocal_approx_fast(rec[:], pd[:])
                    at = work.tile([P, CH], dt_mm, tag="at", bufs=4,
                                   name=f"at{c2}{h2}")
                    nc.vector.tensor_mul(at[:], po[:], rec[:])
                    at_sb.setdefault(c2, []).append(at)
                    if len(at_sb[c2]) == 2 and c2 < 3:
                        for mp_i in range(4):
                            add_filler(g + 4 * mp_i + 6, g,
                                       out_step(c2, mp_i))

            pv_done = 0
            for g in range(NB + 5):
                if g < NB:
                    emit_fillers(g)
                    i, c, h, mb = blocks[g]
                    ps = psum.tile([P, 1024], f32, tag="ps", name=f"s{c}{h}{mb}")
                    nc.tensor.matmul(
                        ps[:, 0:CH],
                        lhsT=qk_sb["k"][h][0:64, ts(mb, P)],
                        rhs=qk_sb["q"][h][0:64, ts(c, CH)],
                    )
                    nc.tensor.matmul(
                        ps[:, CH:1024],
                        lhsT=qk_sb["k"][h][64:P, ts(mb, P)],
                        rhs=qk_sb["q"][h][64:P, ts(c, CH)],
                    )
                    pt = work.tile([P, 1024], dt_mm, tag="pt", bufs=22,
                                   name=f"pt{c}{h}{mb}")
                    nc.scalar.activation(pt[:], ps[:], Exp, scale=SCALE)
                    pts[g] = pt
                while pv_done < NB and pv_done <= g - lag_target(g):
                    process_pv(pv_done, g)
                    pv_done += 1

            # ---- tail: final out-proj chunk ----
            for f in fillers:
                pump(f[2])
            fillers.clear()
            for mp in range(4):
                pump(out_step(3, mp, tail=True))

    nc.compile()
    return nc


def _get_nc():
    if "nc" not in _NC_CACHE:
        _NC_CACHE["nc"] = build_nc(DT_MM_NAME)
    return _NC_CACHE["nc"]


def make_in_maps(x, Wq, bq, Wk, bk, Wv, bv, Wp, bp, dt_mm_name="bfloat16"):
    """Shard full inputs into 8 per-core input maps."""
    f = np.float32
    if dt_mm_name == "bfloat16":
        import ml_dtypes
        mmt = ml_dtypes.bfloat16
    else:
        mmt = np.float32
    x = np.asarray(x, f)
    # kt-interleaved chunk-major x: [4 chunks, 128 partitions, 8*512] so one
    # DMA descriptor per chunk moves 8KB contiguous per partition.
    xTc = []
    for b in range(x.shape[0]):
        xt = np.ascontiguousarray(x[b].T)            # [1024, 2048]
        xTc.append(np.ascontiguousarray(
            xt.reshape(KT, P, NCH, CH).transpose(2, 1, 0, 3).reshape(
                NCH, P, KT * CH)).astype(mmt))
    WqT = np.asarray(Wq, f).T
    WkT = np.asarray(Wk, f).T
    WvT = np.asarray(Wv, f).T
    WpT = np.asarray(Wp, f).T

    def pretile(w):
        return np.ascontiguousarray(
            w.reshape(KT, P, D_LOC).transpose(1, 0, 2).reshape(P, KT * D_LOC)
        ).astype(mmt)

    in_maps = []
    for core in range(N_CORES):
        b, r = divmod(core, 4)
        sl = slice(D_LOC * r, D_LOC * (r + 1))
        bq_l = np.asarray(bq, f)[sl]
        bk_l = np.asarray(bk, f)[sl]
        bqk_l = np.stack([bq_l[0:P], bq_l[P:2 * P],
                          bk_l[0:P], bk_l[P:2 * P]], axis=1)
        in_maps.append({
            "xTc": xTc[b],
            "wqT": pretile(WqT[:, sl]),
            "wkT": pretile(WkT[:, sl]),
            "wvT": pretile(WvT[:, sl]),
            "wpT": np.ascontiguousarray(WpT[sl, :]).astype(mmt),
            "bqk": np.ascontiguousarray(bqk_l).astype(f),
        })
    return in_maps


def assemble_output(results, Wv, bv, Wp, bp):
    """Sum TP partials, transpose back, add folded biases."""
    f = np.float32
    bp_eff = np.asarray(bv, f) @ np.asarray(Wp, f).T + np.asarray(bp, f)
    out = np.empty((2, N_TOK, DIM), f)
    for b in range(2):
        acc = results[4 * b]["outT"].astype(f)
        for r in range(1, 4):
            acc = acc + results[4 * b + r]["outT"].astype(f)
        out[b] = acc.T + bp_eff
    return out


DT_MM_NAME = "bfloat16"


def kernel(x, Wq, bq, Wk, bk, Wv, bv, Wp, bp):
    from concourse.bass_utils import run_bass_kernel_spmd
    nc = _get_nc()
    in_maps = make_in_maps(x, Wq, bq, Wk, bk, Wv, bv, Wp, bp, DT_MM_NAME)
    res = run_bass_kernel_spmd(nc, in_maps, list(range(N_CORES)))
    return assemble_output(res.results, Wv, bv, Wp, bp)
